# revision 11
# baseline (speedup 1.0000x reference)
"""BoundaryLoss kernel for 8 Trainium2 NeuronCores.

Math (exact reformulation of the reference, verified in numpy):
  D_k(m) = k-fold 3x3x3 binary dilation (in-volume only) of mask m
  dist_pos = t       * (4 - (D_1+D_2+D_3)(1-t))
  dist_neg = (1 - t) * (4 - (D_1+D_2+D_3)(t))
  out = sum(sigmoid(logits) * (dist_neg-dist_pos) * valid) / max(sum(valid), 1)

Sharding: 8 cores = batch(2) x H-quarters(4). Each core gets a 72-slice
H-slab (64 owned + 4 halo each side, zero padded at volume edges) in
[H, D, W] order so each h-slice is a [D=96 partitions, W=256 free] tile.

Per h-slice, per dilation round r (both chains fused side by side in the
free dim):
  - W-direction 3-tap max on DVE (2 bf16 tensor_tensor max ops)
  - H+D directions on TensorE: psum += Band3 @ y_r(h+dh), dh in {-1,0,1}
    (Band3 = 96x96 tridiagonal ones -> D-direction 3-tap; the dh
    accumulation in PSUM is the H-direction 3-tap)
  - threshold on ScalarE: x_{r+1} = Sign(psum)   (psum >= 0)
Term pass fuses the signed-distance algebra into scalar_tensor_tensor
ops with a per-partition accum_out reduction; host sums the partials.
"""

import numpy as np

_B, _D, _H, _W = 2, 96, 256, 256
_OWN = 64          # owned h-slices per core
_HALO = 4          # dilation halo
_SLAB = _OWN + 2 * _HALO  # 72

_cache = {}


def _drain_patch(tile_mod, bass_mod, mybir_mod):
    """CoreV3 walrus in this container rejects sync waits on DRAIN; move the
    Tile tail-drain waits onto explicit WAIT instructions."""
    from bass_rust import ScopedClock

    def patched(self, tick_clock, wait_clock):
        nc = self.nc
        nop_inst = nc.sync.nop()
        wait_clock.add_sem_waits(
            nop_inst.ins, ScopedClock({None: tick_clock.global_clock})
        )
        si = nop_inst.ins.sync_info
        waits = list(si.on_wait) if si is not None and si.on_wait else []
        nop_inst.ins.sync_info = mybir_mod.SyncInfo(on_wait=[], on_update=[])
        num2h = {h.num: h for h in self.sems.allocated().values()}
        for w in waits:
            h = num2h.get(w.id) or bass_mod.SemaphoreHandle(name=w.ant_name, num=w.id)
            nc.sync.wait_ge(h, w.wait_value)
        nc.sync.drain()
        nc.all_engine_barrier()
        popped = nc._tile_sem_poison_stack.pop()
        assert popped is self._sem_poison
        nc.clear_and_free_semaphores(list(self.sems.allocated().values()))
        nc.all_engine_barrier()

    tile_mod.TileContext._drain_and_barrier = patched


def _split_sync_waits(nc, mybir_mod, limit=2):
    """Walrus in this container rejects instructions carrying more than
    `limit` sem waits; hoist the overflow onto NoOps on the same engine."""
    n_id = [0]
    for fn in nc.m.functions:
        for bb in fn.blocks:
            out = []
            for inst in bb.instructions:
                si = inst.sync_info
                waits = list(si.on_wait) if si is not None and si.on_wait else []
                if len(waits) > limit:
                    keep = waits[:limit]
                    extra = waits[limit:]
                    for i in range(0, len(extra), limit):
                        nop = mybir_mod.InstNoOp(name=f"wsplit-{n_id[0]}")
                        n_id[0] += 1
                        nop.engine = inst.engine
                        nop.sync_info = mybir_mod.SyncInfo(
                            on_wait=extra[i : i + limit], on_update=[]
                        )
                        out.append(nop)
                    inst.sync_info = mybir_mod.SyncInfo(
                        on_wait=keep, on_update=list(si.on_update or [])
                    )
                out.append(inst)
            bb.instructions = out


def build_program(D=_D, W=_W, own=_OWN, halo=_HALO, num_cores=8):
    """Build the per-core bass program. Returns nc."""
    from contextlib import ExitStack

    import concourse.bass as bass
    import concourse.mybir as mybir
    import concourse.tile as tile

    _drain_patch(tile, bass, mybir)

    dt = mybir.dt
    Alu = mybir.AluOpType
    Act = mybir.ActivationFunctionType

    slab = own + 2 * halo
    Wp = W + 2  # padded W (zero pad col each side)

    nc = bass.Bass()
    # combined chains slab: [:, :, 0, :] = bg (1-t in-volume, 0 outside),
    # [:, :, 1, :] = fg (t)
    ab_in = nc.declare_dram_parameter("ab", [slab, D, 2, W], dt.bfloat16, isOutput=False)
    lg_in = nc.declare_dram_parameter("lg", [own, D, W], dt.float32, isOutput=False)
    vm_in = nc.declare_dram_parameter("vm", [own, D, W], dt.bfloat16, isOutput=False)
    band_in = nc.declare_dram_parameter("band", [D, D], dt.bfloat16, isOutput=False)
    out_dram = nc.declare_dram_parameter("out", [D, own], dt.float32, isOutput=True)

    with tile.TileContext(nc, num_cores=num_cores) as tc, ExitStack() as ctx:
        const_pool = ctx.enter_context(tc.tile_pool(name="const", bufs=1))
        band = const_pool.tile([D, D], dt.bfloat16)
        nc.sync.dma_start(band[:], band_in[:])
        acc = const_pool.tile([D, own], dt.float32)

        x_pools = [
            ctx.enter_context(tc.tile_pool(name=f"x{r}", bufs=b))
            for r, b in enumerate((8, 6, 5, 3))
        ]
        y_pools = [
            ctx.enter_context(tc.tile_pool(name=f"y{r}", bufs=4)) for r in range(3)
        ]
        psum_pool = ctx.enter_context(tc.tile_pool(name="ps", bufs=6, space="PSUM"))
        lg_pool = ctx.enter_context(tc.tile_pool(name="lg", bufs=3))
        vm_pool = ctx.enter_context(tc.tile_pool(name="vm", bufs=3))
        tm_pool = ctx.enter_context(tc.tile_pool(name="tm", bufs=3))

        # rings of python refs: index by absolute slice id
        x = [dict() for _ in range(4)]  # x[r][h] -> tile [D, 2, Wp] bf16
        y = [dict() for _ in range(3)]  # y[r][h] -> tile [D, 2, Wp] bf16

        def w3(xt, pool):
            """y = 3-tap max along W of xt (both chains), zero-padded."""
            yt = pool.tile([D, 2, Wp], dt.bfloat16)
            nc.vector.tensor_tensor(
                yt[:, :, 1 : W + 1], xt[:, :, 0:W], xt[:, :, 1 : W + 1], Alu.max
            )
            nc.vector.tensor_tensor(
                yt[:, :, 1 : W + 1], yt[:, :, 1 : W + 1], xt[:, :, 2 : W + 2], Alu.max
            )
            return yt

        def dilate_hd(r, h):
            """x[r+1][h] = Sign( sum_dh Band3 @ y[r][h+dh] )."""
            ps = psum_pool.tile([D, 2, W], dt.float32)
            for j, dh in enumerate((-1, 0, 1)):
                nc.tensor.matmul(
                    ps[:],
                    band[:],
                    y[r][h + dh][:, :, 1 : W + 1],
                    start=(j == 0),
                    stop=(j == 2),
                )
            xt = x_pools[r + 1].tile([D, 2, Wp], dt.bfloat16)
            if r + 1 < 3:
                # pad cols must be zero (read by the next W3); x3 pads unread
                nc.vector.memset(xt[:, :, 0 : Wp : Wp - 1], 0.0)
            nc.scalar.activation(xt[:, :, 1 : W + 1], ps[:], Act.Sign)
            x[r + 1][h] = xt
            return xt

        for s in range(slab + 3):
            if s < slab:
                # load both chains for slice s: x0 = (bg | fg)
                x0 = x_pools[0].tile([D, 2, Wp], dt.bfloat16)
                nc.vector.memset(x0[:, :, 0 : Wp : Wp - 1], 0.0)
                nc.sync.dma_start(x0[:, :, 1 : W + 1], ab_in[s])
                x[0][s] = x0
                y[0][s] = w3(x0, y_pools[0])
            h1 = s - 1
            if halo - 2 <= h1 <= slab - halo + 1:
                xt = dilate_hd(0, h1)
                y[1][h1] = w3(xt, y_pools[1])
            h2 = s - 2
            if halo - 1 <= h2 <= slab - halo:
                xt = dilate_hd(1, h2)
                y[2][h2] = w3(xt, y_pools[2])
            h3 = s - 3
            if halo <= h3 <= slab - halo - 1:
                dilate_hd(2, h3)
                # ---- term pass for owned slice h3 ----
                o = h3 - halo
                lg = lg_pool.tile([D, W], dt.float32)
                nc.sync.dma_start(lg[:], lg_in[o])
                vm = vm_pool.tile([D, W], dt.bfloat16)
                nc.sync.dma_start(vm[:], vm_in[o])
                probs = tm_pool.tile([D, W], dt.bfloat16, tag="probs")
                nc.scalar.activation(probs[:], lg[:], Act.Sigmoid)
                pv = tm_pool.tile([D, W], dt.bfloat16, tag="pv")
                nc.vector.tensor_tensor(pv[:], probs[:], vm[:], Alu.mult)

                # u = x1 + x2 + x3 (both chains)   [D, 2, W]
                u = tm_pool.tile([D, 2, W], dt.bfloat16, tag="u")
                nc.vector.tensor_tensor(
                    u[:], x[1][h3][:, :, 1 : W + 1], x[2][h3][:, :, 1 : W + 1], Alu.add
                )
                nc.vector.tensor_tensor(
                    u[:], u[:], x[3][h3][:, :, 1 : W + 1], Alu.add
                )
                # v8 = uA + uB ; z = (v8 - 8) * t ; s2 = -uB + z
                # signed = s2 + 4 ; term = (s2 + 4) * pv  (accum over W)
                v8 = tm_pool.tile([D, W], dt.bfloat16, tag="v8")
                nc.vector.tensor_tensor(v8[:], u[:, 0, :], u[:, 1, :], Alu.add)
                z = tm_pool.tile([D, W], dt.bfloat16, tag="z")
                nc.vector.scalar_tensor_tensor(
                    z[:], v8[:], 8.0, x[0][h3][:, 1, 1 : W + 1],
                    Alu.subtract, Alu.mult,
                )
                s2 = tm_pool.tile([D, W], dt.bfloat16, tag="s2")
                nc.vector.scalar_tensor_tensor(
                    s2[:], u[:, 1, :], -1.0, z[:], Alu.mult, Alu.add
                )
                trash = tm_pool.tile([D, W], dt.bfloat16, tag="trash")
                nc.vector.scalar_tensor_tensor(
                    trash[:], s2[:], 4.0, pv[:], Alu.add, Alu.mult,
                    accum_out=acc[:, o : o + 1],
                )

        nc.sync.dma_start(out_dram[:], acc[:])

    _split_sync_waits(nc, mybir, limit=1)
    return nc


def _shard_inputs(logits, targets, valid_mask):
    import ml_dtypes

    bf16 = ml_dtypes.bfloat16
    # reorder to [B][H, D, W]
    t_hdw = targets[:, 0].transpose(0, 2, 1, 3)   # [B, H, D, W]
    lg_hdw = logits[:, 0].transpose(0, 2, 1, 3)
    vm_hdw = valid_mask[:, 0].transpose(0, 2, 1, 3)

    band = np.zeros((_D, _D), dtype=bf16)
    for i in range(_D):
        band[i, max(0, i - 1) : min(_D, i + 2)] = 1
    in_maps = []
    for c in range(8):
        b, q = divmod(c, 4)
        h0 = q * _OWN
        ab = np.zeros((_SLAB, _D, 2, _W), dtype=bf16)
        lo, hi = h0 - _HALO, h0 + _OWN + _HALO
        clo, chi = max(lo, 0), min(hi, _H)
        tt = t_hdw[b, clo:chi]
        ab[clo - lo : chi - lo, :, 0, :] = (1.0 - tt).astype(bf16)
        ab[clo - lo : chi - lo, :, 1, :] = tt.astype(bf16)
        in_maps.append(
            {
                "ab": ab,
                "lg": np.ascontiguousarray(lg_hdw[b, h0 : h0 + _OWN]),
                "vm": vm_hdw[b, h0 : h0 + _OWN].astype(bf16),
                "band": band,
            }
        )
    return in_maps


def kernel(logits, targets, valid_mask):
    from concourse.bass_utils import run_bass_kernel_spmd

    if "nc" not in _cache:
        _cache["nc"] = build_program()
    nc = _cache["nc"]

    in_maps = _shard_inputs(logits, targets, valid_mask)
    core_ids = list(range(8))
    res = run_bass_kernel_spmd(nc, in_maps, core_ids)
    num = np.float64(0.0)
    for i in core_ids:
        num += np.float64(res.results[i]["out"].astype(np.float64).sum())
    den = max(np.float64(valid_mask.sum(dtype=np.float64)), 1.0)
    return np.float32(num / den)


# revision 17
# speedup vs baseline: 1.1497x; 1.1497x over previous
"""BoundaryLoss kernel for 8 Trainium2 NeuronCores.

Math (exact reformulation of the reference, verified in numpy):
  D_k(m) = k-fold 3x3x3 binary dilation (in-volume only) of mask m
  dist_pos = t       * (4 - (D_1+D_2+D_3)(1-t))
  dist_neg = (1 - t) * (4 - (D_1+D_2+D_3)(t))
  out = sum(sigmoid(logits) * (dist_neg-dist_pos) * valid) / max(sum(valid), 1)

Sharding: 8 cores = batch(2) x H-quarters(4). Each core gets a 72-slice
H-slab (64 owned + 4 halo each side, zero padded at volume edges) in
[H, D, W] order so each h-slice is a [D=96 partitions, W=256 free] tile.

Per h-slice, per dilation round r (both chains fused side by side in the
free dim):
  - W-direction 3-tap max on DVE (2 bf16 tensor_tensor max ops)
  - H+D directions on TensorE: psum += Band3 @ y_r(h+dh), dh in {-1,0,1}
    (Band3 = 96x96 tridiagonal ones -> D-direction 3-tap; the dh
    accumulation in PSUM is the H-direction 3-tap)
  - threshold on ScalarE: x_{r+1} = Sign(psum)   (psum >= 0)
Term pass fuses the signed-distance algebra into scalar_tensor_tensor
ops with a per-partition accum_out reduction; host sums the partials.
"""

import numpy as np

_B, _D, _H, _W = 2, 96, 256, 256
_OWN = 64          # owned h-slices per core
_HALO = 4          # dilation halo
_SLAB = _OWN + 2 * _HALO  # 72

_cache = {}


def _drain_patch(tile_mod, bass_mod, mybir_mod):
    """CoreV3 walrus in this container rejects sync waits on DRAIN; move the
    Tile tail-drain waits onto explicit WAIT instructions."""
    from bass_rust import ScopedClock

    def patched(self, tick_clock, wait_clock):
        nc = self.nc
        nop_inst = nc.sync.nop()
        wait_clock.add_sem_waits(
            nop_inst.ins, ScopedClock({None: tick_clock.global_clock})
        )
        si = nop_inst.ins.sync_info
        waits = list(si.on_wait) if si is not None and si.on_wait else []
        nop_inst.ins.sync_info = mybir_mod.SyncInfo(on_wait=[], on_update=[])
        num2h = {h.num: h for h in self.sems.allocated().values()}
        for w in waits:
            h = num2h.get(w.id) or bass_mod.SemaphoreHandle(name=w.ant_name, num=w.id)
            nc.sync.wait_ge(h, w.wait_value)
        nc.sync.drain()
        nc.all_engine_barrier()
        popped = nc._tile_sem_poison_stack.pop()
        assert popped is self._sem_poison
        nc.clear_and_free_semaphores(list(self.sems.allocated().values()))
        nc.all_engine_barrier()

    tile_mod.TileContext._drain_and_barrier = patched


def _split_sync_waits(nc, mybir_mod, limit=2):
    """Walrus in this container rejects instructions carrying more than
    `limit` sem waits; hoist the overflow onto NoOps on the same engine."""
    n_id = [0]
    for fn in nc.m.functions:
        for bb in fn.blocks:
            out = []
            for inst in bb.instructions:
                si = inst.sync_info
                waits = list(si.on_wait) if si is not None and si.on_wait else []
                if len(waits) > limit:
                    keep = waits[:limit]
                    extra = waits[limit:]
                    for i in range(0, len(extra), limit):
                        nop = mybir_mod.InstNoOp(name=f"wsplit-{n_id[0]}")
                        n_id[0] += 1
                        nop.engine = inst.engine
                        nop.sync_info = mybir_mod.SyncInfo(
                            on_wait=extra[i : i + limit], on_update=[]
                        )
                        out.append(nop)
                    inst.sync_info = mybir_mod.SyncInfo(
                        on_wait=keep, on_update=list(si.on_update or [])
                    )
                out.append(inst)
            bb.instructions = out


def build_program(D=_D, W=_W, own=_OWN, halo=_HALO, num_cores=8):
    """Build the per-core bass program. Returns nc."""
    from contextlib import ExitStack

    import concourse.bass as bass
    import concourse.mybir as mybir
    import concourse.tile as tile

    _drain_patch(tile, bass, mybir)

    dt = mybir.dt
    Alu = mybir.AluOpType
    Act = mybir.ActivationFunctionType

    slab = own + 2 * halo
    Wp = W + 2  # padded W (zero pad col each side)

    nc = bass.Bass()
    # combined chains slab: [:, :, 0, :] = bg (1-t in-volume, 0 outside),
    # [:, :, 1, :] = fg (t)
    ab_in = nc.declare_dram_parameter("ab", [slab, D, 2, W], dt.bfloat16, isOutput=False)
    lg_in = nc.declare_dram_parameter("lg", [own, D, W], dt.float32, isOutput=False)
    vm_in = nc.declare_dram_parameter("vm", [own, D, W], dt.bfloat16, isOutput=False)
    band_in = nc.declare_dram_parameter("band", [D, D], dt.bfloat16, isOutput=False)
    # out[:, :own] = per-(D, h) numerator partials; out[:, own:] = denominator
    out_dram = nc.declare_dram_parameter("out", [D, 2 * own], dt.float32, isOutput=True)

    with tile.TileContext(nc, num_cores=num_cores) as tc, ExitStack() as ctx:
        const_pool = ctx.enter_context(tc.tile_pool(name="const", bufs=1))
        band = const_pool.tile([D, D], dt.bfloat16)
        nc.sync.dma_start(band[:], band_in[:])
        acc = const_pool.tile([D, 2 * own], dt.float32)

        x_pools = [
            ctx.enter_context(tc.tile_pool(name=f"x{r}", bufs=b))
            for r, b in enumerate((8, 6, 5, 3))
        ]
        y_pools = [
            ctx.enter_context(tc.tile_pool(name=f"y{r}", bufs=6)) for r in range(3)
        ]
        psum_pool = ctx.enter_context(tc.tile_pool(name="ps", bufs=7, space="PSUM"))
        lg_pool = ctx.enter_context(tc.tile_pool(name="lg", bufs=4))
        vm_pool = ctx.enter_context(tc.tile_pool(name="vm", bufs=4))
        tm_pool = ctx.enter_context(tc.tile_pool(name="tm", bufs=4))

        # rings of python refs: index by absolute slice id
        x = [dict() for _ in range(4)]  # x[r][h] -> tile [D, 2, Wp] bf16
        y = [dict() for _ in range(3)]  # y[r][h] -> tile [D, 2, Wp] bf16

        def w3(xt, pool):
            """y = 3-tap max along W of xt (both chains), zero-padded."""
            yt = pool.tile([D, 2, Wp], dt.bfloat16)
            nc.vector.tensor_tensor(
                yt[:, :, 1 : W + 1], xt[:, :, 0:W], xt[:, :, 1 : W + 1], Alu.max
            )
            nc.vector.tensor_tensor(
                yt[:, :, 1 : W + 1], yt[:, :, 1 : W + 1], xt[:, :, 2 : W + 2], Alu.max
            )
            return yt

        def dilate_hd(r, h):
            """x[r+1][h] = Sign( sum_dh Band3 @ y[r][h+dh] )."""
            ps = psum_pool.tile([D, 2, W], dt.float32)
            for j, dh in enumerate((-1, 0, 1)):
                nc.tensor.matmul(
                    ps[:],
                    band[:],
                    y[r][h + dh][:, :, 1 : W + 1],
                    start=(j == 0),
                    stop=(j == 2),
                )
            xt = x_pools[r + 1].tile([D, 2, Wp], dt.bfloat16)
            if r + 1 < 3:
                # pad cols must be zero (read by the next W3); x3 pads unread
                nc.gpsimd.memset(xt[:, :, 0 : Wp : Wp - 1], 0.0)
            nc.scalar.activation(xt[:, :, 1 : W + 1], ps[:], Act.Sign)
            x[r + 1][h] = xt
            return xt

        for s in range(slab + 3):
            if s < slab:
                # load both chains for slice s: x0 = (bg | fg)
                x0 = x_pools[0].tile([D, 2, Wp], dt.bfloat16)
                nc.gpsimd.memset(x0[:, :, 0 : Wp : Wp - 1], 0.0)
                nc.sync.dma_start(x0[:, :, 1 : W + 1], ab_in[s])
                x[0][s] = x0
                y[0][s] = w3(x0, y_pools[0])
            h1 = s - 1
            if halo - 2 <= h1 <= slab - halo + 1:
                xt = dilate_hd(0, h1)
                y[1][h1] = w3(xt, y_pools[1])
            h2 = s - 2
            if halo - 1 <= h2 <= slab - halo:
                xt = dilate_hd(1, h2)
                y[2][h2] = w3(xt, y_pools[2])
            h3 = s - 3
            if halo <= h3 <= slab - halo - 1:
                dilate_hd(2, h3)
                # ---- term pass for owned slice h3 ----
                o = h3 - halo
                lg = lg_pool.tile([D, W], dt.float32)
                nc.sync.dma_start(lg[:], lg_in[o])
                vm = vm_pool.tile([D, W], dt.bfloat16)
                nc.sync.dma_start(vm[:], vm_in[o])
                probs = tm_pool.tile([D, W], dt.bfloat16, tag="probs")
                nc.scalar.activation(probs[:], lg[:], Act.Sigmoid)
                dtr = tm_pool.tile([D, W], dt.bfloat16, tag="dtr")
                nc.scalar.activation(
                    dtr[:], vm[:], Act.Copy, accum_out=acc[:, own + o : own + o + 1]
                )
                pv = tm_pool.tile([D, W], dt.bfloat16, tag="pv")
                nc.gpsimd.tensor_tensor(pv[:], probs[:], vm[:], Alu.mult)

                # u = x1 + x2 + x3 (both chains)   [D, 2, W]
                u = tm_pool.tile([D, 2, W], dt.bfloat16, tag="u")
                nc.vector.tensor_tensor(
                    u[:], x[1][h3][:, :, 1 : W + 1], x[2][h3][:, :, 1 : W + 1], Alu.add
                )
                nc.vector.tensor_tensor(
                    u[:], u[:], x[3][h3][:, :, 1 : W + 1], Alu.add
                )
                # v8 = uA + uB ; z = (v8 - 8) * t ; s2 = -uB + z
                # signed = s2 + 4 ; term = (s2 + 4) * pv  (accum over W)
                v8 = tm_pool.tile([D, W], dt.bfloat16, tag="v8")
                nc.vector.tensor_tensor(v8[:], u[:, 0, :], u[:, 1, :], Alu.add)
                z = tm_pool.tile([D, W], dt.bfloat16, tag="z")
                nc.vector.scalar_tensor_tensor(
                    z[:], v8[:], 8.0, x[0][h3][:, 1, 1 : W + 1],
                    Alu.subtract, Alu.mult,
                )
                s2 = tm_pool.tile([D, W], dt.bfloat16, tag="s2")
                nc.vector.scalar_tensor_tensor(
                    s2[:], u[:, 1, :], -1.0, z[:], Alu.mult, Alu.add
                )
                trash = tm_pool.tile([D, W], dt.bfloat16, tag="trash")
                nc.vector.scalar_tensor_tensor(
                    trash[:], s2[:], 4.0, pv[:], Alu.add, Alu.mult,
                    accum_out=acc[:, o : o + 1],
                )

        nc.sync.dma_start(out_dram[:], acc[:])

    _split_sync_waits(nc, mybir, limit=1)
    return nc


def _shard_inputs(logits, targets, valid_mask):
    import ml_dtypes

    bf16 = ml_dtypes.bfloat16
    # reorder to [B][H, D, W]
    t_hdw = targets[:, 0].transpose(0, 2, 1, 3)   # [B, H, D, W]
    lg_hdw = logits[:, 0].transpose(0, 2, 1, 3)
    vm_hdw = valid_mask[:, 0].transpose(0, 2, 1, 3)

    band = np.zeros((_D, _D), dtype=bf16)
    for i in range(_D):
        band[i, max(0, i - 1) : min(_D, i + 2)] = 1
    in_maps = []
    for c in range(8):
        b, q = divmod(c, 4)
        h0 = q * _OWN
        ab = np.zeros((_SLAB, _D, 2, _W), dtype=bf16)
        lo, hi = h0 - _HALO, h0 + _OWN + _HALO
        clo, chi = max(lo, 0), min(hi, _H)
        tt = t_hdw[b, clo:chi]
        ab[clo - lo : chi - lo, :, 0, :] = (1.0 - tt).astype(bf16)
        ab[clo - lo : chi - lo, :, 1, :] = tt.astype(bf16)
        in_maps.append(
            {
                "ab": ab,
                "lg": np.ascontiguousarray(lg_hdw[b, h0 : h0 + _OWN]),
                "vm": vm_hdw[b, h0 : h0 + _OWN].astype(bf16),
                "band": band,
            }
        )
    return in_maps


def kernel(logits, targets, valid_mask):
    from concourse.bass_utils import run_bass_kernel_spmd

    if "nc" not in _cache:
        _cache["nc"] = build_program()
    nc = _cache["nc"]

    in_maps = _shard_inputs(logits, targets, valid_mask)
    core_ids = list(range(8))
    res = run_bass_kernel_spmd(nc, in_maps, core_ids)
    num = np.float64(0.0)
    den = np.float64(0.0)
    for i in core_ids:
        o = res.results[i]["out"].astype(np.float64)
        num += o[:, :_OWN].sum()
        den += o[:, _OWN:].sum()
    return np.float32(num / max(den, 1.0))


# revision 21
# speedup vs baseline: 1.1570x; 1.0064x over previous
"""BoundaryLoss kernel for 8 Trainium2 NeuronCores.

Math (exact reformulation of the reference, verified in numpy):
  D_k(m) = k-fold 3x3x3 binary dilation (in-volume only) of mask m
  dist_pos = t       * (4 - (D_1+D_2+D_3)(1-t))
  dist_neg = (1 - t) * (4 - (D_1+D_2+D_3)(t))
  out = sum(sigmoid(logits) * (dist_neg-dist_pos) * valid) / max(sum(valid), 1)

Sharding: 8 cores = batch(2) x H-quarters(4). Each core gets a 72-slice
H-slab (64 owned + 4 halo each side, zero padded at volume edges) in
[H, D, W] order so each h-slice is a [D=96 partitions, W=256 free] tile.
h-slices are processed in PAIRS to halve instruction counts.

Per slice pair, per dilation round r (both chains + both slices fused in
the free dim):
  - W-direction 3-tap max on DVE (2 bf16 tensor_tensor max ops)
  - H+D directions on TensorE: psum[si] += Band3 @ y_r(h+dh), dh in
    {-1,0,1} (Band3 = tridiagonal ones -> D-direction 3-tap; the dh
    accumulation in PSUM is the H-direction 3-tap)
  - threshold on ScalarE: x_{r+1} = Sign(psum)   (psum >= 0)
Term pass fuses the signed-distance algebra into scalar_tensor_tensor
ops with a per-partition accum_out reduction; host sums the partials.
"""

import numpy as np

_B, _D, _H, _W = 2, 96, 256, 256
_OWN = 64          # owned h-slices per core
_HALO = 4          # dilation halo
_SLAB = _OWN + 2 * _HALO  # 72

_cache = {}


def _drain_patch(tile_mod, bass_mod, mybir_mod):
    """CoreV3 walrus in this container rejects sync waits on DRAIN; move the
    Tile tail-drain waits onto explicit WAIT instructions."""
    from bass_rust import ScopedClock

    def patched(self, tick_clock, wait_clock):
        nc = self.nc
        nop_inst = nc.sync.nop()
        wait_clock.add_sem_waits(
            nop_inst.ins, ScopedClock({None: tick_clock.global_clock})
        )
        si = nop_inst.ins.sync_info
        waits = list(si.on_wait) if si is not None and si.on_wait else []
        nop_inst.ins.sync_info = mybir_mod.SyncInfo(on_wait=[], on_update=[])
        num2h = {h.num: h for h in self.sems.allocated().values()}
        for w in waits:
            h = num2h.get(w.id) or bass_mod.SemaphoreHandle(name=w.ant_name, num=w.id)
            nc.sync.wait_ge(h, w.wait_value)
        nc.sync.drain()
        nc.all_engine_barrier()
        popped = nc._tile_sem_poison_stack.pop()
        assert popped is self._sem_poison
        nc.clear_and_free_semaphores(list(self.sems.allocated().values()))
        nc.all_engine_barrier()

    tile_mod.TileContext._drain_and_barrier = patched


def _split_sync_waits(nc, mybir_mod, limit=1):
    """Walrus in this container rejects instructions carrying more than
    `limit` sem waits; hoist the overflow onto NoOps on the same engine."""
    n_id = [0]
    for fn in nc.m.functions:
        for bb in fn.blocks:
            out = []
            for inst in bb.instructions:
                si = inst.sync_info
                waits = list(si.on_wait) if si is not None and si.on_wait else []
                if len(waits) > limit:
                    keep = waits[:limit]
                    extra = waits[limit:]
                    for i in range(0, len(extra), limit):
                        nop = mybir_mod.InstNoOp(name=f"wsplit-{n_id[0]}")
                        n_id[0] += 1
                        nop.engine = inst.engine
                        nop.sync_info = mybir_mod.SyncInfo(
                            on_wait=extra[i : i + limit], on_update=[]
                        )
                        out.append(nop)
                    inst.sync_info = mybir_mod.SyncInfo(
                        on_wait=keep, on_update=list(si.on_update or [])
                    )
                out.append(inst)
            bb.instructions = out


def build_program(D=_D, W=_W, own=_OWN, halo=_HALO, num_cores=8, split_waits=True):
    """Build the per-core bass program (paired h-slices). Returns nc."""
    from contextlib import ExitStack

    import concourse.bass as bass
    import concourse.mybir as mybir
    import concourse.tile as tile

    _drain_patch(tile, bass, mybir)

    dt = mybir.dt
    Alu = mybir.AluOpType
    Act = mybir.ActivationFunctionType

    slab = own + 2 * halo
    assert slab % 2 == 0 and halo % 2 == 0 and own % 2 == 0
    NP = slab // 2          # total pairs
    P0 = halo // 2          # first owned pair
    P1 = NP - halo // 2     # one past last owned pair
    npair_own = own // 2
    Wp = W + 2  # padded W (zero pad col each side)

    nc = bass.Bass()
    # combined chains slab: [:, :, 0, :] = bg (1-t in-volume, 0 outside),
    # [:, :, 1, :] = fg (t)
    ab_in = nc.declare_dram_parameter("ab", [slab, D, 2, W], dt.bfloat16, isOutput=False)
    lg_in = nc.declare_dram_parameter("lg", [own, D, W], dt.float32, isOutput=False)
    vm_in = nc.declare_dram_parameter("vm", [own, D, W], dt.bfloat16, isOutput=False)
    band_in = nc.declare_dram_parameter("band", [D, D], dt.bfloat16, isOutput=False)
    # out[:, :own//2] = per-(D, pair) numerator partials; [:, own//2:] = denom
    out_dram = nc.declare_dram_parameter("out", [D, own], dt.float32, isOutput=True)

    with tile.TileContext(nc, num_cores=num_cores) as tc, ExitStack() as ctx:
        const_pool = ctx.enter_context(tc.tile_pool(name="const", bufs=1))
        band = const_pool.tile([D, D], dt.bfloat16)
        nc.sync.dma_start(band[:], band_in[:])
        acc = const_pool.tile([D, own], dt.float32)
        yzero = const_pool.tile([D, 2, Wp], dt.bfloat16)
        nc.vector.memset(yzero[:], 0.0)

        x_pools = [
            ctx.enter_context(tc.tile_pool(name=f"x{r}", bufs=b))
            for r, b in enumerate((6, 5, 4, 3))
        ]
        y_pools = [
            ctx.enter_context(tc.tile_pool(name=f"y{r}", bufs=4)) for r in range(3)
        ]
        psum_pool = ctx.enter_context(
            tc.tile_pool(name="ps", bufs=3, space="PSUM")
        )
        lg_pool = ctx.enter_context(tc.tile_pool(name="lg", bufs=3))
        vm_pool = ctx.enter_context(tc.tile_pool(name="vm", bufs=3))
        tm_pool = ctx.enter_context(tc.tile_pool(name="tm", bufs=3))

        x = [dict() for _ in range(4)]  # x[r][k] -> pair tile [D, 2, 2, Wp]
        y = [dict() for _ in range(3)]  # y[r][k] -> pair tile [D, 2, 2, Wp]

        def yslice(r, s):
            """rhs view of slice s of round r (zero outside slab)."""
            if s < 0 or s >= slab:
                return yzero[:, :, 1 : W + 1]
            return y[r][s // 2][:, s % 2, :, 1 : W + 1]

        def w3(xt, pool):
            yt = pool.tile([D, 2, 2, Wp], dt.bfloat16)
            nc.vector.tensor_tensor(
                yt[:, :, :, 1 : W + 1],
                xt[:, :, :, 0:W],
                xt[:, :, :, 1 : W + 1],
                Alu.max,
            )
            nc.vector.tensor_tensor(
                yt[:, :, :, 1 : W + 1],
                yt[:, :, :, 1 : W + 1],
                xt[:, :, :, 2 : W + 2],
                Alu.max,
            )
            return yt

        def dilate_hd(r, k):
            """x[r+1][k] = Sign( sum_dh Band3 @ y[r][slices of pair k +dh] )."""
            ps = psum_pool.tile([D, 2, 2, W], dt.float32)
            for si in range(2):
                s = 2 * k + si
                for j, dh in enumerate((-1, 0, 1)):
                    nc.tensor.matmul(
                        ps[:, si],
                        band[:],
                        yslice(r, s + dh),
                        start=(j == 0),
                        stop=(j == 2),
                    )
            xt = x_pools[r + 1].tile([D, 2, 2, Wp], dt.bfloat16)
            if r + 1 < 3:
                # pad cols must be zero (read by the next W3); x3 pads unread
                nc.vector.memset(xt[:, :, :, 0 : Wp : Wp - 1], 0.0)
            nc.scalar.activation(xt[:, :, :, 1 : W + 1], ps[:], Act.Sign)
            x[r + 1][k] = xt
            return xt

        for k in range(NP + 3):
            if k < NP:
                # load both chains for pair k: x0 = (bg | fg) x 2 slices
                x0 = x_pools[0].tile([D, 2, 2, Wp], dt.bfloat16)
                nc.vector.memset(x0[:, :, :, 0 : Wp : Wp - 1], 0.0)
                for si in range(2):
                    nc.sync.dma_start(
                        x0[:, si, :, 1 : W + 1], ab_in[2 * k + si]
                    )
                x[0][k] = x0
                y[0][k] = w3(x0, y_pools[0])
            k1 = k - 1
            if 0 <= k1 < NP:
                y[1][k1] = w3(dilate_hd(0, k1), y_pools[1])
            k2 = k - 2
            if P0 - 1 <= k2 <= P1:
                y[2][k2] = w3(dilate_hd(1, k2), y_pools[2])
            k3 = k - 3
            if P0 <= k3 < P1:
                dilate_hd(2, k3)
                # ---- term pass for owned pair k3 ----
                o = k3 - P0  # owned pair index
                lg = lg_pool.tile([D, 2, W], dt.float32)
                nc.sync.dma_start(
                    lg[:], lg_in[2 * o : 2 * o + 2].transpose([1, 0, 2])
                )
                vm = vm_pool.tile([D, 2, W], dt.bfloat16)
                nc.sync.dma_start(
                    vm[:], vm_in[2 * o : 2 * o + 2].transpose([1, 0, 2])
                )
                probs = tm_pool.tile([D, 2, W], dt.bfloat16, tag="probs")
                nc.scalar.activation(probs[:], lg[:], Act.Sigmoid)
                dtr = tm_pool.tile([D, 2, W], dt.bfloat16, tag="dtr")
                nc.scalar.activation(
                    dtr[:], vm[:], Act.Copy,
                    accum_out=acc[:, npair_own + o : npair_own + o + 1],
                )
                pv = tm_pool.tile([D, 2, W], dt.bfloat16, tag="pv")
                nc.vector.tensor_tensor(pv[:], probs[:], vm[:], Alu.mult)

                # u = x1 + x2 + x3 (both chains, both slices)  [D, 2, 2, W]
                u = tm_pool.tile([D, 2, 2, W], dt.bfloat16, tag="u")
                nc.vector.tensor_tensor(
                    u[:],
                    x[1][k3][:, :, :, 1 : W + 1],
                    x[2][k3][:, :, :, 1 : W + 1],
                    Alu.add,
                )
                nc.vector.tensor_tensor(
                    u[:], u[:], x[3][k3][:, :, :, 1 : W + 1], Alu.add
                )
                # v8 = uA + uB ; z = (v8 - 8) * t ; s2 = -uB + z
                # signed = s2 + 4 ; term = (s2 + 4) * pv  (accum over W)
                v8 = tm_pool.tile([D, 2, W], dt.bfloat16, tag="v8")
                nc.vector.tensor_tensor(
                    v8[:], u[:, :, 0, :], u[:, :, 1, :], Alu.add
                )
                z = tm_pool.tile([D, 2, W], dt.bfloat16, tag="z")
                nc.vector.scalar_tensor_tensor(
                    z[:], v8[:], 8.0, x[0][k3][:, :, 1, 1 : W + 1],
                    Alu.subtract, Alu.mult,
                )
                s2 = tm_pool.tile([D, 2, W], dt.bfloat16, tag="s2")
                nc.vector.scalar_tensor_tensor(
                    s2[:], u[:, :, 1, :], -1.0, z[:], Alu.mult, Alu.add
                )
                trash = tm_pool.tile([D, 2, W], dt.bfloat16, tag="trash")
                nc.vector.scalar_tensor_tensor(
                    trash[:], s2[:], 4.0, pv[:], Alu.add, Alu.mult,
                    accum_out=acc[:, o : o + 1],
                )

        nc.sync.dma_start(out_dram[:], acc[:])

    if split_waits:
        _split_sync_waits(nc, mybir, limit=1)
    return nc


def _shard_inputs(logits, targets, valid_mask):
    import ml_dtypes

    bf16 = ml_dtypes.bfloat16
    # reorder to [B][H, D, W]
    t_hdw = targets[:, 0].transpose(0, 2, 1, 3)   # [B, H, D, W]
    lg_hdw = logits[:, 0].transpose(0, 2, 1, 3)
    vm_hdw = valid_mask[:, 0].transpose(0, 2, 1, 3)

    band = np.zeros((_D, _D), dtype=bf16)
    for i in range(_D):
        band[i, max(0, i - 1) : min(_D, i + 2)] = 1
    in_maps = []
    for c in range(8):
        b, q = divmod(c, 4)
        h0 = q * _OWN
        ab = np.zeros((_SLAB, _D, 2, _W), dtype=bf16)
        lo, hi = h0 - _HALO, h0 + _OWN + _HALO
        clo, chi = max(lo, 0), min(hi, _H)
        tt = t_hdw[b, clo:chi]
        ab[clo - lo : chi - lo, :, 0, :] = (1.0 - tt).astype(bf16)
        ab[clo - lo : chi - lo, :, 1, :] = tt.astype(bf16)
        in_maps.append(
            {
                "ab": ab,
                "lg": np.ascontiguousarray(lg_hdw[b, h0 : h0 + _OWN]),
                "vm": vm_hdw[b, h0 : h0 + _OWN].astype(bf16),
                "band": band,
            }
        )
    return in_maps


def kernel(logits, targets, valid_mask):
    from concourse.bass_utils import run_bass_kernel_spmd

    if "nc" not in _cache:
        _cache["nc"] = build_program()
    nc = _cache["nc"]

    in_maps = _shard_inputs(logits, targets, valid_mask)
    core_ids = list(range(8))
    res = run_bass_kernel_spmd(nc, in_maps, core_ids)
    num = np.float64(0.0)
    den = np.float64(0.0)
    half = _OWN // 2
    for i in core_ids:
        o = res.results[i]["out"].astype(np.float64)
        num += o[:, :half].sum()
        den += o[:, half:].sum()
    return np.float32(num / max(den, 1.0))


# revision 28
# speedup vs baseline: 1.2203x; 1.0547x over previous
"""BoundaryLoss kernel for 8 Trainium2 NeuronCores.

Math (exact reformulation of the reference, verified in numpy):
  D_k(m) = k-fold 3x3x3 binary dilation (in-volume only) of mask m
  dist_pos = t       * (4 - (D_1+D_2+D_3)(1-t))
  dist_neg = (1 - t) * (4 - (D_1+D_2+D_3)(t))
  out = sum(sigmoid(logits) * (dist_neg-dist_pos) * valid) / max(sum(valid), 1)

Sharding: 8 cores = batch(2) x H-quarters(4). Each core gets a 72-slice
H-slab (64 owned + 4 halo each side, zero padded at volume edges) in
[H, D, W] order so each h-slice is a [D=96 partitions, W=256 free] tile.
h-slices are processed in PAIRS to halve instruction counts.

Per slice pair, per dilation round r (both chains + both slices fused in
the free dim):
  - W-direction 3-tap max on DVE (2 bf16 tensor_tensor max ops)
  - H+D directions on TensorE: psum[si] += Band3 @ y_r(h+dh), dh in
    {-1,0,1} (Band3 = tridiagonal ones -> D-direction 3-tap; the dh
    accumulation in PSUM is the H-direction 3-tap)
  - threshold on ScalarE: x_{r+1} = Sign(psum)   (psum >= 0)
Term pass fuses the signed-distance algebra into scalar_tensor_tensor
ops with a per-partition accum_out reduction; host sums the partials.
"""

import numpy as np

_B, _D, _H, _W = 2, 96, 256, 256
_OWN = 64          # owned h-slices per core
_HALO = 4          # dilation halo
_SLAB = _OWN + 2 * _HALO  # 72

_cache = {}


def _drain_patch(tile_mod, bass_mod, mybir_mod):
    """CoreV3 walrus in this container rejects sync waits on DRAIN; move the
    Tile tail-drain waits onto explicit WAIT instructions."""
    from bass_rust import ScopedClock

    def patched(self, tick_clock, wait_clock):
        nc = self.nc
        nop_inst = nc.sync.nop()
        wait_clock.add_sem_waits(
            nop_inst.ins, ScopedClock({None: tick_clock.global_clock})
        )
        si = nop_inst.ins.sync_info
        waits = list(si.on_wait) if si is not None and si.on_wait else []
        nop_inst.ins.sync_info = mybir_mod.SyncInfo(on_wait=[], on_update=[])
        num2h = {h.num: h for h in self.sems.allocated().values()}
        for w in waits:
            h = num2h.get(w.id) or bass_mod.SemaphoreHandle(name=w.ant_name, num=w.id)
            nc.sync.wait_ge(h, w.wait_value)
        nc.sync.drain()
        nc.all_engine_barrier()
        popped = nc._tile_sem_poison_stack.pop()
        assert popped is self._sem_poison
        nc.clear_and_free_semaphores(list(self.sems.allocated().values()))
        nc.all_engine_barrier()

    tile_mod.TileContext._drain_and_barrier = patched


def _split_sync_waits(nc, mybir_mod, limit=1):
    """Walrus in this container rejects instructions carrying more than
    `limit` sem waits; hoist the overflow onto NoOps on the same engine."""
    n_id = [0]
    for fn in nc.m.functions:
        for bb in fn.blocks:
            out = []
            for inst in bb.instructions:
                si = inst.sync_info
                waits = list(si.on_wait) if si is not None and si.on_wait else []
                if len(waits) > limit:
                    keep = waits[:limit]
                    extra = waits[limit:]
                    for i in range(0, len(extra), limit):
                        nop = mybir_mod.InstNoOp(name=f"wsplit-{n_id[0]}")
                        n_id[0] += 1
                        nop.engine = inst.engine
                        nop.sync_info = mybir_mod.SyncInfo(
                            on_wait=extra[i : i + limit], on_update=[]
                        )
                        out.append(nop)
                    inst.sync_info = mybir_mod.SyncInfo(
                        on_wait=keep, on_update=list(si.on_update or [])
                    )
                out.append(inst)
            bb.instructions = out


def build_program(D=_D, W=_W, own=_OWN, halo=_HALO, num_cores=8, split_waits=True):
    """Build the per-core bass program (paired h-slices). Returns nc."""
    from contextlib import ExitStack

    import concourse.bass as bass
    import concourse.mybir as mybir
    import concourse.tile as tile

    _drain_patch(tile, bass, mybir)

    dt = mybir.dt
    Alu = mybir.AluOpType
    Act = mybir.ActivationFunctionType

    slab = own + 2 * halo
    assert slab % 2 == 0 and halo % 2 == 0 and own % 2 == 0
    NP = slab // 2          # total pairs
    P0 = halo // 2          # first owned pair
    P1 = NP - halo // 2     # one past last owned pair
    npair_own = own // 2
    Wp = W + 2  # padded W (zero pad col each side)

    nc = bass.Bass()
    # combined chains slab: [:, :, 0, :] = bg (1-t in-volume, 0 outside),
    # [:, :, 1, :] = fg (t); W-padded with zero cols at 0 and W+1
    ab_in = nc.declare_dram_parameter(
        "ab", [slab, D, 2, Wp], dt.bfloat16, isOutput=False
    )
    lg_in = nc.declare_dram_parameter("lg", [own, D, W], dt.float32, isOutput=False)
    vm_in = nc.declare_dram_parameter("vm", [own, D, W], dt.bfloat16, isOutput=False)
    band_in = nc.declare_dram_parameter("band", [D, D], dt.bfloat16, isOutput=False)
    # out[:, :own//2] = per-(D, pair) numerator partials; [:, own//2:] = denom
    out_dram = nc.declare_dram_parameter("out", [D, own], dt.float32, isOutput=True)

    with tile.TileContext(nc, num_cores=num_cores) as tc, ExitStack() as ctx:
        const_pool = ctx.enter_context(tc.tile_pool(name="const", bufs=1))
        band = const_pool.tile([D, D], dt.bfloat16)
        nc.sync.dma_start(band[:], band_in[:])
        acc = const_pool.tile([D, own], dt.float32)
        yzero = const_pool.tile([D, 2, Wp], dt.bfloat16)
        nc.vector.memset(yzero[:], 0.0)

        x_pools = [
            ctx.enter_context(tc.tile_pool(name=f"x{r}", bufs=b))
            for r, b in enumerate((8, 6, 5, 4))
        ]
        y_pools = [
            ctx.enter_context(tc.tile_pool(name=f"y{r}", bufs=6)) for r in range(3)
        ]
        psum_pool = ctx.enter_context(
            tc.tile_pool(name="ps", bufs=4, space="PSUM")
        )
        lg_pool = ctx.enter_context(tc.tile_pool(name="lg", bufs=4))
        vm_pool = ctx.enter_context(tc.tile_pool(name="vm", bufs=4))
        tm_pool = ctx.enter_context(tc.tile_pool(name="tm", bufs=4))

        x = [dict() for _ in range(4)]  # x[r][k] -> pair tile [D, 2, 2, Wp]
        y = [dict() for _ in range(3)]  # y[r][k] -> pair tile [D, 2, 2, Wp]

        def yslice(r, s):
            """rhs view of slice s of round r (zero outside slab)."""
            if s < 0 or s >= slab:
                return yzero[:, :, 1 : W + 1]
            return y[r][s // 2][:, s % 2, :, 1 : W + 1]

        def w3(xt, pool):
            yt = pool.tile([D, 2, 2, Wp], dt.bfloat16)
            nc.vector.tensor_tensor(
                yt[:, :, :, 1 : W + 1],
                xt[:, :, :, 0:W],
                xt[:, :, :, 1 : W + 1],
                Alu.max,
            )
            nc.vector.tensor_tensor(
                yt[:, :, :, 1 : W + 1],
                yt[:, :, :, 1 : W + 1],
                xt[:, :, :, 2 : W + 2],
                Alu.max,
            )
            return yt

        def dilate_hd(r, k):
            """x[r+1][k] = Sign( sum_dh Band3 @ y[r][slices of pair k +dh] )."""
            ps = psum_pool.tile([D, 2, 2, W], dt.float32)
            for si in range(2):
                s = 2 * k + si
                for j, dh in enumerate((-1, 0, 1)):
                    nc.tensor.matmul(
                        ps[:, si],
                        band[:],
                        yslice(r, s + dh),
                        start=(j == 0),
                        stop=(j == 2),
                    )
            xt = x_pools[r + 1].tile([D, 2, 2, Wp], dt.bfloat16)
            if r + 1 < 3:
                # pad cols must be zero (read by the next W3); x3 pads unread
                nc.scalar.activation(
                    xt[:, :, :, 0 : Wp : Wp - 1], ps[:, :, :, 0:2], Act.Sign,
                    scale=0.0,
                )
            nc.scalar.activation(xt[:, :, :, 1 : W + 1], ps[:], Act.Sign)
            x[r + 1][k] = xt
            return xt

        for k in range(NP + 3):
            if k < NP:
                # load both chains for pair k: x0 = (bg | fg) x 2 slices
                x0 = x_pools[0].tile([D, 2, 2, Wp], dt.bfloat16)
                for si in range(2):
                    nc.sync.dma_start(x0[:, si], ab_in[2 * k + si])
                x[0][k] = x0
                y[0][k] = w3(x0, y_pools[0])
            k1 = k - 1
            if 0 <= k1 < NP:
                y[1][k1] = w3(dilate_hd(0, k1), y_pools[1])
            k2 = k - 2
            if P0 - 1 <= k2 <= P1:
                y[2][k2] = w3(dilate_hd(1, k2), y_pools[2])
            k3 = k - 3
            if P0 <= k3 < P1:
                dilate_hd(2, k3)
                # ---- term pass for owned pair k3 ----
                o = k3 - P0  # owned pair index
                lg = lg_pool.tile([D, 2, W], dt.float32)
                nc.sync.dma_start(
                    lg[:], lg_in[2 * o : 2 * o + 2].transpose([1, 0, 2])
                )
                vm = vm_pool.tile([D, 2, W], dt.bfloat16)
                nc.sync.dma_start(
                    vm[:], vm_in[2 * o : 2 * o + 2].transpose([1, 0, 2])
                )
                probs = tm_pool.tile([D, 2, W], dt.bfloat16, tag="probs")
                nc.scalar.activation(probs[:], lg[:], Act.Sigmoid)
                dtr = tm_pool.tile([D, 2, W], dt.bfloat16, tag="dtr")
                nc.scalar.activation(
                    dtr[:], vm[:], Act.Copy,
                    accum_out=acc[:, npair_own + o : npair_own + o + 1],
                )
                pv = tm_pool.tile([D, 2, W], dt.bfloat16, tag="pv")
                nc.vector.tensor_tensor(pv[:], probs[:], vm[:], Alu.mult)

                # u = x1 + x2 + x3 (both chains, both slices)  [D, 2, 2, W]
                u = tm_pool.tile([D, 2, 2, W], dt.bfloat16, tag="u")
                nc.vector.tensor_tensor(
                    u[:],
                    x[1][k3][:, :, :, 1 : W + 1],
                    x[2][k3][:, :, :, 1 : W + 1],
                    Alu.add,
                )
                nc.vector.tensor_tensor(
                    u[:], u[:], x[3][k3][:, :, :, 1 : W + 1], Alu.add
                )
                # v8 = uA + uB ; z = (v8 - 8) * t ; s2 = -uB + z
                # signed = s2 + 4 ; term = (s2 + 4) * pv  (accum over W)
                v8 = tm_pool.tile([D, 2, W], dt.bfloat16, tag="v8")
                nc.vector.tensor_tensor(
                    v8[:], u[:, :, 0, :], u[:, :, 1, :], Alu.add
                )
                z = tm_pool.tile([D, 2, W], dt.bfloat16, tag="z")
                nc.vector.scalar_tensor_tensor(
                    z[:], v8[:], 8.0, x[0][k3][:, :, 1, 1 : W + 1],
                    Alu.subtract, Alu.mult,
                )
                s2 = tm_pool.tile([D, 2, W], dt.bfloat16, tag="s2")
                nc.vector.scalar_tensor_tensor(
                    s2[:], u[:, :, 1, :], -1.0, z[:], Alu.mult, Alu.add
                )
                trash = tm_pool.tile([D, 2, W], dt.bfloat16, tag="trash")
                nc.vector.scalar_tensor_tensor(
                    trash[:], s2[:], 4.0, pv[:], Alu.add, Alu.mult,
                    accum_out=acc[:, o : o + 1],
                )

        nc.sync.dma_start(out_dram[:], acc[:])

    if split_waits:
        _split_sync_waits(nc, mybir, limit=1)
    return nc


def _shard_inputs(logits, targets, valid_mask):
    import ml_dtypes

    bf16 = ml_dtypes.bfloat16
    # reorder to [B][H, D, W]
    t_hdw = targets[:, 0].transpose(0, 2, 1, 3)   # [B, H, D, W]
    lg_hdw = logits[:, 0].transpose(0, 2, 1, 3)
    vm_hdw = valid_mask[:, 0].transpose(0, 2, 1, 3)

    band = np.zeros((_D, _D), dtype=bf16)
    for i in range(_D):
        band[i, max(0, i - 1) : min(_D, i + 2)] = 1
    in_maps = []
    for c in range(8):
        b, q = divmod(c, 4)
        h0 = q * _OWN
        ab = np.zeros((_SLAB, _D, 2, _W + 2), dtype=bf16)
        lo, hi = h0 - _HALO, h0 + _OWN + _HALO
        clo, chi = max(lo, 0), min(hi, _H)
        tt = t_hdw[b, clo:chi]
        ab[clo - lo : chi - lo, :, 0, 1 : _W + 1] = (1.0 - tt).astype(bf16)
        ab[clo - lo : chi - lo, :, 1, 1 : _W + 1] = tt.astype(bf16)
        in_maps.append(
            {
                "ab": ab,
                "lg": np.ascontiguousarray(lg_hdw[b, h0 : h0 + _OWN]),
                "vm": vm_hdw[b, h0 : h0 + _OWN].astype(bf16),
                "band": band,
            }
        )
    return in_maps


def kernel(logits, targets, valid_mask):
    from concourse.bass_utils import run_bass_kernel_spmd

    if "nc" not in _cache:
        _cache["nc"] = build_program()
    nc = _cache["nc"]

    in_maps = _shard_inputs(logits, targets, valid_mask)
    core_ids = list(range(8))
    res = run_bass_kernel_spmd(nc, in_maps, core_ids)
    num = np.float64(0.0)
    den = np.float64(0.0)
    half = _OWN // 2
    for i in core_ids:
        o = res.results[i]["out"].astype(np.float64)
        num += o[:, :half].sum()
        den += o[:, half:].sum()
    return np.float32(num / max(den, 1.0))
